# revision 16
# baseline (speedup 1.0000x reference)
"""Trainium2 Bass kernel for NoSharingGraphConv.

out[b,w,m] = sum_{h,n} x[b,h,n] * adj[h,w] * W[h,w,n,m] + bias[m]
  B=4096, N=17 (graph nodes), FIN=FOUT=256.

Sharding (8 NeuronCores): 4 batch groups x 2 out-feature halves.
Core c handles batch rows [bg*1024, (bg+1)*1024) and out features
[mh*128, (mh+1)*128), bg = c>>1, mh = c&1. This halves the per-core W
stream (18.9MB bf16) vs pure batch-parallel while keeping the PE work
perfectly balanced (1156 matmul-equivalents of [128x128]x[128x512]).

Device kernel (per core):
  - adj is folded into W on the HOST (W_adj = adj[h,w]*W[h,w,n,m], cast
    to bf16 during the swizzle): no on-device DVE scaling, no adj DMA.
  - Cold start is latency-engineered around the measured DMA behavior
    (each queue ring sustains ~210GB/s; ~420 aggregate):
      * The first 4 w-slabs are host-packed H-INTERLEAVED into one
        "cold block" [p, h, kc, w4, m'] streamed on the SP ring in 10
        h-slices, so the h-slices every chain needs first arrive first.
      * x^T bh0 streams on the ACT ring in parallel (6 pieces, small
        first piece).
      * Pass A runs the 4 cold slabs c-outer (4 interleaved PSUM
        chains): the PE consumes each arriving (h, xt-chunk) slice 4x
        and stays gapless from ~10.5us at the HBM arrival rate.
      * Tapered warmup matmuls (memset-fed junk) ramp the PE clock
        during the preamble+DMA window without blocking the queue.
  - Steady state: per (w, batch-half) 34 accumulating matmuls into one
    PSUM bank; ACT evacuates with the per-partition bias add (fp32).
  - The last group is split into 4 sequential 128-free chains so the
    final evac+DMA exposure at the tail is ~0.5us instead of ~2.3.
  - Device writes out_t [17, 128, 1024] (w, m', b); host permutes back.
"""

import sys

if "/opt/trn_rl_repo" not in sys.path:
    sys.path.insert(0, "/opt/trn_rl_repo")

import numpy as np

B, N, FIN, FOUT = 4096, 17, 256, 256
NC = 8
NBG = 4  # batch groups
BS = B // NBG  # 1024 batch rows per core
MH = FOUT // 2  # 128 out features per core
KCH = N * FIN // 128  # 34 contraction chunks of 128
NBH = BS // 512  # 2 batch halves (matmul free dim 512)
NW0 = 4  # slabs packed into the h-interleaved cold block

# cold-block DMA h-slices: fine-grained early for a fast first matmul
# and to ride the DMA-ring ramp without stalling the in-order PE queue
COLD_SPLITS = ((0, 1), (1, 2), (2, 3), (3, 4), (4, 5), (5, 6), (6, 8),
               (8, 10), (10, 12), (12, 14), (14, N))
# xt DMA split points (chunks of 128 contraction rows); bh0/bh1 pieces
# are issued ALTERNATING so both halves stream in together for the
# fused 8-chain cold pass
XT_SPLITS = ((0, 1), (1, 3), (3, 6), (6, 10), (10, 15), (15, 21),
             (21, 28), (28, KCH))

# final-group column splits: small last chains shrink the tail exposure
CHAIN_SPLITS = ((0, 128), (128, 256), (256, 384), (384, 512))

WARM_BIG = 4  # 512-free warmup matmuls (ramp the clock)
WARM_SMALL = 8  # 128-free warmup matmuls (fine-grained tail)

_CACHE = {}


def _build_module():
    import concourse.mybir as mybir
    import concourse.tile as tile
    from concourse import bacc

    f32 = mybir.dt.float32
    bf16 = mybir.dt.bfloat16

    nc = bacc.Bacc("TRN2", target_bir_lowering=False)

    # bf16 inputs: halves the dominant W DMA stream, halves the x^T
    # prologue load, and enables the PE fast-weight-load path.
    # host-prepared, batch-half-major, partition-major:
    #   xt[bh, p, c, b'] = bf16(x[bh*512+b', h, 2p+kc]), c = 2h+kc
    xt_d = nc.dram_tensor("xt", [NBH, 128, KCH, 512], bf16, kind="ExternalInput")
    # cold block, h-interleaved over the first NW0 slabs:
    #   wc[p, h, kc, w, m'] = bf16(adj[h,w] * W[h, w, 2p+kc, mh*128+m'])
    wc_d = nc.dram_tensor("w_cold", [128, N, 2, NW0, MH], bf16, kind="ExternalInput")
    # steady slabs, w = NW0..N-1:
    #   w_sw[w-NW0, p, h, kc, m'] = bf16(adj[h,w] * W[h, w, 2p+kc, mh*128+m'])
    w_d = nc.dram_tensor(
        "w_sw", [N - NW0, 128, N, 2, MH], bf16, kind="ExternalInput"
    )
    b_d = nc.dram_tensor("b", [MH], f32, kind="ExternalInput")
    o_d = nc.dram_tensor("out_t", [N, MH, BS], f32, kind="ExternalOutput")

    with tile.TileContext(nc) as tc:
        with (
            tc.tile_pool(name="const", bufs=1) as const,
            tc.tile_pool(name="wslab", bufs=5) as wpool,
            tc.tile_pool(name="obuf", bufs=4) as opool,
            tc.tile_pool(name="psum", bufs=8, space="PSUM") as psum,
        ):
            # PE warm-up: junk matmuls during the prologue DMA window
            # release the HAM clock gate (1.2 -> 2.4 GHz) before the
            # real matmuls start. memset-fed (gpsimd), no DMA
            # dependency. Tapered so the PE frees up the moment real
            # work is ready.
            warm = const.tile([1, 512], bf16)
            nc.gpsimd.memset(warm[:], 0.0)
            warm_ps = psum.tile([1, 512], f32, tag="ps")
            for _ in range(WARM_BIG):
                nc.tensor.matmul(
                    warm_ps[:], lhsT=warm[:, 0:1], rhs=warm[:], start=True, stop=True
                )
            for _ in range(WARM_SMALL):
                nc.tensor.matmul(
                    warm_ps[:, 0:128],
                    lhsT=warm[:, 0:1],
                    rhs=warm[:, 0:128],
                    start=True,
                    stop=True,
                )

            # cold block on the SP ring, h-sliced (arrival order == the
            # order the interleaved chains consume it); the very first
            # h is split by kc so the first matmul gate is half-size
            cold_sb = const.tile([128, N, 2, NW0, MH], bf16)
            for kc in range(2):
                nc.sync.dma_start(
                    cold_sb[:, 0:1, kc : kc + 1].rearrange(
                        "p h kc w m -> p (h kc w m)"
                    ),
                    wc_d[:, 0:1, kc : kc + 1].rearrange(
                        "p h kc w m -> p (h kc w m)"
                    ),
                )
            for h0, h1 in COLD_SPLITS[1:]:
                nc.sync.dma_start(
                    cold_sb[:, h0:h1].rearrange("p h kc w m -> p (h kc w m)"),
                    wc_d[:, h0:h1].rearrange("p h kc w m -> p (h kc w m)"),
                )

            # resident x^T, bh-major: every DMA fully contiguous per
            # partition; ACT ring, parallel to the SP ring. bh0/bh1
            # pieces alternate so the fused cold pass (which consumes
            # both halves chunk-by-chunk) is never starved on either.
            xt_sb = const.tile([128, NBH, KCH, 512], bf16)
            for c0, c1 in XT_SPLITS:
                for bh in range(NBH):
                    nc.scalar.dma_start(
                        xt_sb[:, bh, c0:c1, :], xt_d[bh, :, c0:c1, :]
                    )

            # bias half on partitions: bias_sb[p, 0] = b[mh*128 + p].
            # 128x4B descriptors are SLOW (~4us of ring time), so issue
            # it after the cold block (needed only at the first evac,
            # ~40us in) and before the steady slab stream.
            bias_sb = const.tile([128, 1], f32)
            nc.sync.dma_start(bias_sb[:], b_d[:][:, None])

            def evac(ps, w, bh):
                ot = opool.tile([128, 512], f32, tag="ot")
                nc.scalar.activation(
                    ot[:],
                    ps[:],
                    mybir.ActivationFunctionType.Identity,
                    bias=bias_sb[:, 0:1],
                )
                nc.scalar.dma_start(o_d[w, :, bh * 512 : (bh + 1) * 512], ot[:])

            # Fused cold pass over slabs 0..NW0-1, BOTH batch halves,
            # c-outer (c = 2h+kc): each arriving W h-slice feeds
            # 2*NW0 matmuls and each xt chunk NW0, so PE consumption
            # outpaces arrival ~1.7-2x even while the DMA rings ramp.
            # Uses all 8 PSUM banks as accumulation chains. Emission
            # is blocked per xt piece, all bh0-chain work before bh1's,
            # so the (later-issued) bh1 piece DMA gets a half-piece
            # window before the in-order PE queue needs it.
            pss = [
                psum.tile([128, 512], f32, tag="ps", name=f"ps_cold_{i}_{bh}")
                for i in range(NW0)
                for bh in range(NBH)
            ]
            for c0, c1 in XT_SPLITS:
                for bh in range(NBH):
                    for c in range(c0, c1):
                        h, kc = divmod(c, 2)
                        for i in range(NW0):
                            nc.tensor.matmul(
                                pss[2 * i + bh][:],
                                lhsT=cold_sb[:, h, kc, i, :],
                                rhs=xt_sb[:, bh, c, :],
                                start=(c == 0),
                                stop=(c == KCH - 1),
                            )
            for i in range(NW0):
                for bh in range(NBH):
                    evac(pss[2 * i + bh], i, bh)

            def load_slab(w):
                # one fully-contiguous 1.1MB slab read (SP ring)
                wt = wpool.tile([128, N, 2, MH], bf16, tag="wslab")
                nc.sync.dma_start(
                    wt[:].rearrange("p h kc m -> p (h kc m)"),
                    w_d[w - NW0].rearrange("p h kc m -> p (h kc m)"),
                )
                return wt

            def mm_group(wt, w, bh):
                ps = psum.tile([128, 512], f32, tag="ps")
                for c in range(KCH):
                    h, kc = divmod(c, 2)
                    nc.tensor.matmul(
                        ps[:],
                        lhsT=wt[:, h, kc, :],
                        rhs=xt_sb[:, bh, c, :],
                        start=(c == 0),
                        stop=(c == KCH - 1),
                    )
                evac(ps, w, bh)

            # steady state: slab w prefetches while w-1 computes
            for w in range(NW0, N):
                wt = load_slab(w)
                mm_group(wt, w, 0)
                if w < N - 1:
                    mm_group(wt, w, 1)
                else:
                    # last group: sequential narrow chains so the
                    # final ACT+DMA exposure is one small tile
                    for q0, q1 in CHAIN_SPLITS:
                        ps = psum.tile([128, 128], f32, tag="ps")
                        for c in range(KCH):
                            h, kc = divmod(c, 2)
                            nc.tensor.matmul(
                                ps[:, 0 : q1 - q0],
                                lhsT=wt[:, h, kc, :],
                                rhs=xt_sb[:, 1, c, q0:q1],
                                start=(c == 0),
                                stop=(c == KCH - 1),
                            )
                        ot = opool.tile([128, 128], f32, tag="ot_small")
                        nc.scalar.activation(
                            ot[:, 0 : q1 - q0],
                            ps[:, 0 : q1 - q0],
                            mybir.ActivationFunctionType.Identity,
                            bias=bias_sb[:, 0:1],
                        )
                        nc.scalar.dma_start(
                            o_d[w, :, 512 + q0 : 512 + q1], ot[:, 0 : q1 - q0]
                        )

    nc.compile()
    return nc


def _get_module():
    if "nc" not in _CACHE:
        _CACHE["nc"] = _build_module()
    return _CACHE["nc"]


def kernel(x, adj, W, b, _trace=False):
    from concourse.bass_utils import run_bass_kernel_spmd

    x = np.ascontiguousarray(np.asarray(x, dtype=np.float32))
    adj = np.ascontiguousarray(np.asarray(adj, dtype=np.float32))
    W = np.ascontiguousarray(np.asarray(W, dtype=np.float32))
    b = np.ascontiguousarray(np.asarray(b, dtype=np.float32))

    nc = _get_module()

    import ml_dtypes

    # adj folded into W on the host (fp32 product, single bf16 round)
    Wa = W * adj[:, :, None, None]
    w_cold = []  # [p, h, kc, w4, m'] for w in 0..NW0-1
    w_sw = []  # [w-NW0, p, h, kc, m'] for w in NW0..N-1
    for mh in range(2):
        wh = Wa[:, :, :, mh * MH : (mh + 1) * MH]  # [h, w, n, m']
        wr = wh.reshape(N, N, FIN // 2, 2, MH)  # (h, w, p, kc, m')
        w_cold.append(
            np.ascontiguousarray(
                wr[:, :NW0].transpose(2, 0, 3, 1, 4)  # (p, h, kc, w, m')
                .astype(ml_dtypes.bfloat16)
            )
        )
        w_sw.append(
            np.ascontiguousarray(
                wr[:, NW0:].transpose(1, 2, 0, 3, 4)  # (w, p, h, kc, m')
                .astype(ml_dtypes.bfloat16)
            )
        )

    xt_by_bg = []
    for bg in range(NBG):
        xs = x[bg * BS : (bg + 1) * BS]  # [BS, N, FIN]
        # xt[bh, p, c, b'] = bf16(x[bh*512+b', h, 2p+kc]), c = 2h+kc
        xr = xs.reshape(NBH, 512, N, FIN // 2, 2)  # (bh, b', h, p, kc)
        xt_by_bg.append(
            np.ascontiguousarray(
                xr.transpose(0, 3, 2, 4, 1)  # (bh, p, h, kc, b')
                .reshape(NBH, 128, KCH, 512)
                .astype(ml_dtypes.bfloat16)
            )
        )

    in_maps = []
    for c in range(NC):
        bg, mh = divmod(c, 2)
        in_maps.append(
            {
                "xt": xt_by_bg[bg],
                "w_cold": w_cold[mh],
                "w_sw": w_sw[mh],
                "b": b[mh * MH : (mh + 1) * MH].copy(),
            }
        )

    res = run_bass_kernel_spmd(nc, in_maps, list(range(NC)), trace=_trace)
    _CACHE["last_result"] = res

    out = np.empty((B, N, FOUT), dtype=np.float32)
    for c in range(NC):
        bg, mh = divmod(c, 2)
        ot = res.results[c]["out_t"]  # [17, 128, 1024] = (w, m', b)
        out[bg * BS : (bg + 1) * BS, :, mh * MH : (mh + 1) * MH] = ot.transpose(
            2, 0, 1
        )
    return out


# revision 17
# speedup vs baseline: 1.0055x; 1.0055x over previous
"""Trainium2 Bass kernel for NoSharingGraphConv.

out[b,w,m] = sum_{h,n} x[b,h,n] * adj[h,w] * W[h,w,n,m] + bias[m]
  B=4096, N=17 (graph nodes), FIN=FOUT=256.

Sharding (8 NeuronCores): 4 batch groups x 2 out-feature halves.
Core c handles batch rows [bg*1024, (bg+1)*1024) and out features
[mh*128, (mh+1)*128), bg = c>>1, mh = c&1. This halves the per-core W
stream (18.9MB bf16) vs pure batch-parallel while keeping the PE work
perfectly balanced (1156 matmul-equivalents of [128x128]x[128x512]).

Device kernel (per core):
  - adj is folded into W on the HOST (W_adj = adj[h,w]*W[h,w,n,m], cast
    to bf16 during the swizzle): no on-device DVE scaling, no adj DMA.
  - Cold start is latency-engineered around the measured DMA behavior
    (each queue ring sustains ~210GB/s; ~420 aggregate):
      * The first 4 w-slabs are host-packed H-INTERLEAVED into one
        "cold block" [p, h, kc, w4, m'] streamed on the SP ring in 10
        h-slices, so the h-slices every chain needs first arrive first.
      * x^T bh0 streams on the ACT ring in parallel (6 pieces, small
        first piece).
      * Pass A runs the 4 cold slabs c-outer (4 interleaved PSUM
        chains): the PE consumes each arriving (h, xt-chunk) slice 4x
        and stays gapless from ~10.5us at the HBM arrival rate.
      * Tapered warmup matmuls (memset-fed junk) ramp the PE clock
        during the preamble+DMA window without blocking the queue.
  - Steady state: per (w, batch-half) 34 accumulating matmuls into one
    PSUM bank; ACT evacuates with the per-partition bias add (fp32).
  - The last group is split into 4 sequential 128-free chains so the
    final evac+DMA exposure at the tail is ~0.5us instead of ~2.3.
  - Device writes out_t [17, 128, 1024] (w, m', b); host permutes back.
"""

import sys

if "/opt/trn_rl_repo" not in sys.path:
    sys.path.insert(0, "/opt/trn_rl_repo")

import numpy as np

B, N, FIN, FOUT = 4096, 17, 256, 256
NC = 8
NBG = 4  # batch groups
BS = B // NBG  # 1024 batch rows per core
MH = FOUT // 2  # 128 out features per core
KCH = N * FIN // 128  # 34 contraction chunks of 128
NBH = BS // 512  # 2 batch halves (matmul free dim 512)
NW0 = 4  # slabs packed into the h-interleaved cold block

# cold-block DMA h-slices: fine-grained early for a fast first matmul
# and to ride the DMA-ring ramp without stalling the in-order PE queue
COLD_SPLITS = ((0, 1), (1, 2), (2, 3), (3, 4), (4, 5), (5, 6), (6, 8),
               (8, 10), (10, 12), (12, 14), (14, N))
# xt DMA split points (chunks of 128 contraction rows); bh0/bh1 pieces
# are issued ALTERNATING so both halves stream in together for the
# fused 8-chain cold pass
XT_SPLITS = ((0, 1), (1, 3), (3, 6), (6, 10), (10, 15), (15, 21),
             (21, 28), (28, KCH))

# final-group column splits: small last chains shrink the tail exposure
CHAIN_SPLITS = ((0, 128), (128, 256), (256, 384), (384, 512))

WARM_BIG = 4  # 512-free warmup matmuls (ramp the clock)
WARM_SMALL = 5  # 128-free warmup matmuls (fine-grained tail)

_CACHE = {}


def _build_module():
    import concourse.mybir as mybir
    import concourse.tile as tile
    from concourse import bacc

    f32 = mybir.dt.float32
    bf16 = mybir.dt.bfloat16

    nc = bacc.Bacc("TRN2", target_bir_lowering=False)

    # bf16 inputs: halves the dominant W DMA stream, halves the x^T
    # prologue load, and enables the PE fast-weight-load path.
    # host-prepared, batch-half-major, partition-major:
    #   xt[bh, p, c, b'] = bf16(x[bh*512+b', h, 2p+kc]), c = 2h+kc
    xt_d = nc.dram_tensor("xt", [NBH, 128, KCH, 512], bf16, kind="ExternalInput")
    # cold block, h-interleaved over the first NW0 slabs:
    #   wc[p, h, kc, w, m'] = bf16(adj[h,w] * W[h, w, 2p+kc, mh*128+m'])
    wc_d = nc.dram_tensor("w_cold", [128, N, 2, NW0, MH], bf16, kind="ExternalInput")
    # steady slabs, w = NW0..N-1:
    #   w_sw[w-NW0, p, h, kc, m'] = bf16(adj[h,w] * W[h, w, 2p+kc, mh*128+m'])
    w_d = nc.dram_tensor(
        "w_sw", [N - NW0, 128, N, 2, MH], bf16, kind="ExternalInput"
    )
    b_d = nc.dram_tensor("b", [MH], f32, kind="ExternalInput")
    o_d = nc.dram_tensor("out_t", [N, MH, BS], f32, kind="ExternalOutput")

    with tile.TileContext(nc) as tc:
        with (
            tc.tile_pool(name="const", bufs=1) as const,
            tc.tile_pool(name="wslab", bufs=5) as wpool,
            tc.tile_pool(name="obuf", bufs=4) as opool,
            tc.tile_pool(name="psum", bufs=8, space="PSUM") as psum,
        ):
            # PE warm-up: junk matmuls during the prologue DMA window
            # release the HAM clock gate (1.2 -> 2.4 GHz) before the
            # real matmuls start. memset-fed (gpsimd), no DMA
            # dependency. Tapered so the PE frees up the moment real
            # work is ready.
            warm = const.tile([1, 512], bf16)
            nc.gpsimd.memset(warm[:], 0.0)
            warm_ps = psum.tile([1, 512], f32, tag="ps")
            for _ in range(WARM_BIG):
                nc.tensor.matmul(
                    warm_ps[:], lhsT=warm[:, 0:1], rhs=warm[:], start=True, stop=True
                )
            for _ in range(WARM_SMALL):
                nc.tensor.matmul(
                    warm_ps[:, 0:128],
                    lhsT=warm[:, 0:1],
                    rhs=warm[:, 0:128],
                    start=True,
                    stop=True,
                )

            # cold block on the SP ring, h-sliced (arrival order == the
            # order the interleaved chains consume it); the very first
            # h is split by kc so the first matmul gate is half-size
            cold_sb = const.tile([128, N, 2, NW0, MH], bf16)
            for kc in range(2):
                nc.sync.dma_start(
                    cold_sb[:, 0:1, kc : kc + 1].rearrange(
                        "p h kc w m -> p (h kc w m)"
                    ),
                    wc_d[:, 0:1, kc : kc + 1].rearrange(
                        "p h kc w m -> p (h kc w m)"
                    ),
                )
            for h0, h1 in COLD_SPLITS[1:]:
                nc.sync.dma_start(
                    cold_sb[:, h0:h1].rearrange("p h kc w m -> p (h kc w m)"),
                    wc_d[:, h0:h1].rearrange("p h kc w m -> p (h kc w m)"),
                )

            # resident x^T, bh-major: every DMA fully contiguous per
            # partition; ACT ring, parallel to the SP ring. bh0/bh1
            # pieces alternate so the fused cold pass (which consumes
            # both halves chunk-by-chunk) is never starved on either.
            xt_sb = const.tile([128, NBH, KCH, 512], bf16)
            for c0, c1 in XT_SPLITS:
                for bh in range(NBH):
                    nc.scalar.dma_start(
                        xt_sb[:, bh, c0:c1, :], xt_d[bh, :, c0:c1, :]
                    )

            # bias half on partitions: bias_sb[p, 0] = b[mh*128 + p].
            # 128x4B descriptors are SLOW (~4us of ring time), so issue
            # it after the cold block (needed only at the first evac,
            # ~40us in) and before the steady slab stream.
            bias_sb = const.tile([128, 1], f32)
            nc.sync.dma_start(bias_sb[:], b_d[:][:, None])

            def evac(ps, w, bh):
                ot = opool.tile([128, 512], f32, tag="ot")
                nc.scalar.activation(
                    ot[:],
                    ps[:],
                    mybir.ActivationFunctionType.Identity,
                    bias=bias_sb[:, 0:1],
                )
                nc.scalar.dma_start(o_d[w, :, bh * 512 : (bh + 1) * 512], ot[:])

            # Fused cold pass over slabs 0..NW0-1, BOTH batch halves,
            # c-outer (c = 2h+kc): each arriving W h-slice feeds
            # 2*NW0 matmuls and each xt chunk NW0, so PE consumption
            # outpaces arrival ~1.7-2x even while the DMA rings ramp.
            # Uses all 8 PSUM banks as accumulation chains. Emission
            # is blocked per xt piece, all bh0-chain work before bh1's,
            # so the (later-issued) bh1 piece DMA gets a half-piece
            # window before the in-order PE queue needs it.
            pss = [
                psum.tile([128, 512], f32, tag="ps", name=f"ps_cold_{i}_{bh}")
                for i in range(NW0)
                for bh in range(NBH)
            ]
            for c0, c1 in XT_SPLITS:
                for bh in range(NBH):
                    for c in range(c0, c1):
                        h, kc = divmod(c, 2)
                        for i in range(NW0):
                            nc.tensor.matmul(
                                pss[2 * i + bh][:],
                                lhsT=cold_sb[:, h, kc, i, :],
                                rhs=xt_sb[:, bh, c, :],
                                start=(c == 0),
                                stop=(c == KCH - 1),
                            )
            for i in range(NW0):
                for bh in range(NBH):
                    evac(pss[2 * i + bh], i, bh)

            def load_slab(w):
                # one fully-contiguous 1.1MB slab read (SP ring)
                wt = wpool.tile([128, N, 2, MH], bf16, tag="wslab")
                nc.sync.dma_start(
                    wt[:].rearrange("p h kc m -> p (h kc m)"),
                    w_d[w - NW0].rearrange("p h kc m -> p (h kc m)"),
                )
                return wt

            def mm_group(wt, w, bh):
                ps = psum.tile([128, 512], f32, tag="ps")
                for c in range(KCH):
                    h, kc = divmod(c, 2)
                    nc.tensor.matmul(
                        ps[:],
                        lhsT=wt[:, h, kc, :],
                        rhs=xt_sb[:, bh, c, :],
                        start=(c == 0),
                        stop=(c == KCH - 1),
                    )
                evac(ps, w, bh)

            # steady state: slab w prefetches while w-1 computes
            for w in range(NW0, N):
                wt = load_slab(w)
                mm_group(wt, w, 0)
                if w < N - 1:
                    mm_group(wt, w, 1)
                else:
                    # last group: sequential narrow chains so the
                    # final ACT+DMA exposure is one small tile
                    for q0, q1 in CHAIN_SPLITS:
                        ps = psum.tile([128, 128], f32, tag="ps")
                        for c in range(KCH):
                            h, kc = divmod(c, 2)
                            nc.tensor.matmul(
                                ps[:, 0 : q1 - q0],
                                lhsT=wt[:, h, kc, :],
                                rhs=xt_sb[:, 1, c, q0:q1],
                                start=(c == 0),
                                stop=(c == KCH - 1),
                            )
                        ot = opool.tile([128, 128], f32, tag="ot_small")
                        nc.scalar.activation(
                            ot[:, 0 : q1 - q0],
                            ps[:, 0 : q1 - q0],
                            mybir.ActivationFunctionType.Identity,
                            bias=bias_sb[:, 0:1],
                        )
                        nc.scalar.dma_start(
                            o_d[w, :, 512 + q0 : 512 + q1], ot[:, 0 : q1 - q0]
                        )

    nc.compile()
    return nc


def _get_module():
    if "nc" not in _CACHE:
        _CACHE["nc"] = _build_module()
    return _CACHE["nc"]


def kernel(x, adj, W, b, _trace=False):
    from concourse.bass_utils import run_bass_kernel_spmd

    x = np.ascontiguousarray(np.asarray(x, dtype=np.float32))
    adj = np.ascontiguousarray(np.asarray(adj, dtype=np.float32))
    W = np.ascontiguousarray(np.asarray(W, dtype=np.float32))
    b = np.ascontiguousarray(np.asarray(b, dtype=np.float32))

    nc = _get_module()

    import ml_dtypes

    # adj folded into W on the host (fp32 product, single bf16 round)
    Wa = W * adj[:, :, None, None]
    w_cold = []  # [p, h, kc, w4, m'] for w in 0..NW0-1
    w_sw = []  # [w-NW0, p, h, kc, m'] for w in NW0..N-1
    for mh in range(2):
        wh = Wa[:, :, :, mh * MH : (mh + 1) * MH]  # [h, w, n, m']
        wr = wh.reshape(N, N, FIN // 2, 2, MH)  # (h, w, p, kc, m')
        w_cold.append(
            np.ascontiguousarray(
                wr[:, :NW0].transpose(2, 0, 3, 1, 4)  # (p, h, kc, w, m')
                .astype(ml_dtypes.bfloat16)
            )
        )
        w_sw.append(
            np.ascontiguousarray(
                wr[:, NW0:].transpose(1, 2, 0, 3, 4)  # (w, p, h, kc, m')
                .astype(ml_dtypes.bfloat16)
            )
        )

    xt_by_bg = []
    for bg in range(NBG):
        xs = x[bg * BS : (bg + 1) * BS]  # [BS, N, FIN]
        # xt[bh, p, c, b'] = bf16(x[bh*512+b', h, 2p+kc]), c = 2h+kc
        xr = xs.reshape(NBH, 512, N, FIN // 2, 2)  # (bh, b', h, p, kc)
        xt_by_bg.append(
            np.ascontiguousarray(
                xr.transpose(0, 3, 2, 4, 1)  # (bh, p, h, kc, b')
                .reshape(NBH, 128, KCH, 512)
                .astype(ml_dtypes.bfloat16)
            )
        )

    in_maps = []
    for c in range(NC):
        bg, mh = divmod(c, 2)
        in_maps.append(
            {
                "xt": xt_by_bg[bg],
                "w_cold": w_cold[mh],
                "w_sw": w_sw[mh],
                "b": b[mh * MH : (mh + 1) * MH].copy(),
            }
        )

    res = run_bass_kernel_spmd(nc, in_maps, list(range(NC)), trace=_trace)
    _CACHE["last_result"] = res

    out = np.empty((B, N, FOUT), dtype=np.float32)
    for c in range(NC):
        bg, mh = divmod(c, 2)
        ot = res.results[c]["out_t"]  # [17, 128, 1024] = (w, m', b)
        out[bg * BS : (bg + 1) * BS, :, mh * MH : (mh + 1) * MH] = ot.transpose(
            2, 0, 1
        )
    return out
